# revision 38
# baseline (speedup 1.0000x reference)
"""AdaFace loss on 8 TRN2 NeuronCores — class-parallel margin softmax.

Sharding: class dim split 12500/core. Host pre-normalizes weight rows and
casts to fp8 in k-major DoubleRow layout [128, 2(kk), 2(o), 12500]; the
device streams W from HBM (6.4MB/core) via group-aligned chunks on the
sync queue. The matmul keeps the (transposed, normalized, fp8) embeddings
stationary and streams W: psum[128 batch, 2048 classes] accumulates K=512
in 2 DoubleRow matmuls per 512-class sub-chunk. Each psum group is
consumed by ACT (exp with accum_out -> per-sample sumexp partials) and DVE
(Schraudolph fast-exp on the tail columns) in parallel.

The margin/label path is kept off the sweep engines: batch-norm stats use
gpsimd partition_all_reduce (no PE matmul), the margin polynomial and the
label-cosine chain run on gpsimd from an f32 row gather, and the few ops
that must touch DVE/ACT are emitted mid-sweep at points those engines
reach only after the inputs are ready (so the in-order queues never
stall). A tiny dummy AllReduce fires at t~0 to absorb the cross-core
rendezvous + first-collective setup; the single data AllReduce at the end
carries sumexp+corr and label logits together.
"""
import math
import numpy as np

NCORES = 8
C, D, N = 100000, 512, 512
CLOC = C // NCORES            # 12500
SUB = 512                     # classes per matmul / psum bank
GRP = 4 * SUB                 # classes per psum tile (4 banks)
NGRP_FULL = CLOC // GRP       # 6 full groups
GRP_LAST = CLOC - NGRP_FULL * GRP   # 212
NGRP = NGRP_FULL + 1          # 7
NTILE = N // 128              # 4 batch tiles
SCALEC = 30.0
MARGIN = 0.4
HCONST = 0.333
FP8S = 16.0                   # fp8 scaling for both operands
S30 = SCALEC / (FP8S * FP8S)  # activation scale: psum = 256*cos
EXP30 = float(np.exp(np.float32(30.0)))
# Schraudolph fast-exp: exp(y) ~= bitcast_f32(int(y*FEA + FEB)); for the
# DVE-consumed columns y = S30*psum - 30, so i = psum*(FEA*S30) + (FEB-30*FEA)
FEA = 12102203.161561485      # 2^23/ln(2)
FEB = 1064866805.0
DVE_A = FEA * S30
DVE_B = FEB - 30.0 * FEA
ASPLIT = 1536                 # cols per group on ACT (3 psum banks); rest on DVE

_cache = {}


def _build():
    import concourse.bass as bass
    import concourse.bacc as bacc
    import concourse.mybir as mybir
    import concourse.tile as tile
    import concourse.bass_isa as bass_isa
    from contextlib import ExitStack

    f32 = mybir.dt.float32
    bf16 = mybir.dt.bfloat16
    fp8 = mybir.dt.float8e4
    i32 = mybir.dt.int32
    AF = mybir.ActivationFunctionType
    OP = mybir.AluOpType
    X = mybir.AxisListType.X

    nc = bacc.Bacc("TRN2", target_bir_lowering=False, debug=False,
                   num_devices=NCORES)
    _c30 = nc.alloc_sbuf_tensor("const-f32-neg30", [128, 1], f32)
    nc.gpsimd.memset(_c30.ap(), -30.0)
    nc.const_aps.aps[(f32, -30.0)] = _c30.ap()
    nc.all_engine_barrier()

    wt8_d = nc.dram_tensor("wt8", [128, 2, 2, CLOC], fp8, kind="ExternalInput")
    wrows_d = nc.dram_tensor("wrows", [CLOC, D], f32, kind="ExternalInput")
    emb_d = nc.dram_tensor("emb", [N, D], f32, kind="ExternalInput")
    labidx_d = nc.dram_tensor("labidx", [128, NTILE], i32, kind="ExternalInput")
    valid_d = nc.dram_tensor("valid", [128, NTILE], f32, kind="ExternalInput")
    identbf_d = nc.dram_tensor("identbf", [128, 128], bf16, kind="ExternalInput")
    onesf_d = nc.dram_tensor("onesf", [128, 1], f32, kind="ExternalInput")
    out_d = nc.dram_tensor("out", [1, 1], f32, kind="ExternalOutput")

    def grp_range(g):
        c0 = g * GRP
        return c0, (GRP if g < NGRP_FULL else GRP_LAST)

    with tile.TileContext(nc) as tc, ExitStack() as ctx:
        constp = ctx.enter_context(tc.tile_pool(name="const", bufs=1))
        scrp = ctx.enter_context(tc.tile_pool(name="scratch", bufs=2))
        actp = ctx.enter_context(tc.tile_pool(name="actout", bufs=2))
        smallp = ctx.enter_context(tc.tile_pool(name="small", bufs=2))
        pmain = ctx.enter_context(tc.tile_pool(name="pmain", bufs=2, space="PSUM"))
        dramp = ctx.enter_context(tc.tile_pool(name="dram", bufs=1, space="DRAM"))

        gp = nc.gpsimd

        # ---- sync queue (hardware DGE, fast): tiny consts, embeddings,
        # then the W chunks. The gpsimd software-DGE queue is far too slow
        # for the embedding tiles.
        idx_sb = constp.tile([128, NTILE], i32, tag="idx")
        nc.sync.dma_start(out=idx_sb[:], in_=labidx_d[:, :])
        ident_bf = constp.tile([128, 128], bf16, tag="identbf")
        nc.sync.dma_start(out=ident_bf[:], in_=identbf_d[:, :])
        ones_f = constp.tile([128, 1], f32, tag="onesf")
        nc.sync.dma_start(out=ones_f[:], in_=onesf_d[:, :])
        valid_sb = constp.tile([128, NTILE], f32, tag="valid")
        nc.sync.dma_start(out=valid_sb[:], in_=valid_d[:, :])
        emb_all = constp.tile([128, NTILE, D], f32, tag="emball")
        nc.sync.dma_start(
            out=emb_all[:],
            in_=emb_d[:, :].rearrange("(j p) d -> p j d", p=128))
        emb_t = [emb_all[:, j, :] for j in range(NTILE)]

        w8 = constp.tile([128, 2, 2, CLOC], fp8, tag="w8")
        for g in range(NGRP):
            c0, w = grp_range(g)
            nc.sync.dma_start(out=w8[:, :, :, c0:c0 + w],
                              in_=wt8_d[:, :, :, c0:c0 + w])

        # ---- gpsimd queue: dummy rendezvous AR, embeddings (parallel with
        # the sync queue's W stream), then the label gather ----
        warm_sb = smallp.tile([128, 1], f32, tag="warm")
        gp.memset(warm_sb[:], 0.0)
        warm_in = dramp.tile([128, 1], f32, tag="warmin")
        warm_out = dramp.tile([128, 1], f32, tag="warmout")
        gp.dma_start(out=warm_in[:], in_=warm_sb[:])
        gp.collective_compute(
            "AllReduce", mybir.AluOpType.add,
            replica_groups=[list(range(NCORES))],
            ins=[warm_in.opt()], outs=[warm_out.opt()])

        wlab_t = []
        for j in range(NTILE):
            wl = constp.tile([128, D], f32, tag=f"wlab{j}")
            gp.indirect_dma_start(
                out=wl[:], out_offset=None, in_=wrows_d[:, :],
                in_offset=bass.IndirectOffsetOnAxis(ap=idx_sb[:, j:j + 1],
                                                    axis=0))
            wlab_t.append(wl)

        def rsqrt(eng, x_ap, y_ap, t_ap, iters=2):
            """y = 1/sqrt(x) via bitcast seed + Newton (x > 0)."""
            xi = x_ap.bitcast(i32)
            yi = y_ap.bitcast(i32)
            eng.tensor_scalar(out=yi, in0=xi, scalar1=1, scalar2=None,
                              op0=OP.arith_shift_right)
            eng.tensor_scalar(out=yi, in0=yi, scalar1=-1,
                              scalar2=0x5f3759df, op0=OP.mult, op1=OP.add)
            for _ in range(iters):
                eng.tensor_tensor(out=t_ap, in0=x_ap, in1=y_ap, op=OP.mult)
                eng.tensor_tensor(out=t_ap, in0=t_ap, in1=y_ap, op=OP.mult)
                eng.tensor_scalar(out=t_ap, in0=t_ap, scalar1=-0.5,
                                  scalar2=1.5, op0=OP.mult, op1=OP.add)
                eng.tensor_tensor(out=y_ap, in0=y_ap, in1=t_ap, op=OP.mult)

        # ---- embedding prep (DVE), pair-phased so j0's chain starts as
        # soon as its data lands. embT8 is one [128, 4(k4), 512(n)] fp8
        # tile; each j gets 4 PE transposes into one psum tile and a single
        # strided ACT cast.
        norms2_b = constp.tile([128, NTILE], f32, tag="norms2")
        invn_b = constp.tile([128, NTILE], f32, tag="invn")
        invn16_b = constp.tile([128, NTILE], f32, tag="invn16")
        embT8 = constp.tile([128, 4, N], fp8, tag="embT8")
        for jp in range(2):
            j0, j1 = 2 * jp, 2 * jp + 1
            for j in (j0, j1):
                scr = scrp.tile([128, D], f32, tag="sq")
                nc.vector.scalar_tensor_tensor(
                    out=scr[:], in0=emb_t[j][:], scalar=1.0, in1=emb_t[j][:],
                    op0=OP.mult, op1=OP.mult, accum_out=norms2_b[:, j:j + 1])
            tmp_b = scrp.tile([128, 2], f32, tag="tmpb")
            rsqrt(nc.vector, norms2_b[:, j0:j1 + 1], invn_b[:, j0:j1 + 1],
                  tmp_b[:])
            nc.vector.tensor_scalar_mul(invn16_b[:, j0:j1 + 1],
                                        invn_b[:, j0:j1 + 1], FP8S)
            for j in (j0, j1):
                e8 = scrp.tile([128, D], bf16, tag="e8")
                nc.vector.tensor_scalar_mul(e8[:], emb_t[j][:],
                                            invn16_b[:, j:j + 1])
                pst = pmain.tile([128, 4, 128], bf16, tag="psB")
                for k4 in range(4):
                    nc.tensor.transpose(out=pst[:, k4, :],
                                        in_=e8[:, k4 * 128:(k4 + 1) * 128],
                                        identity=ident_bf[:])
                nc.scalar.copy(out=embT8[:, :, j * 128:(j + 1) * 128],
                               in_=pst[:])

        norms_b = constp.tile([128, NTILE], f32, tag="norms")
        nc.vector.tensor_tensor(out=norms_b[:], in0=norms2_b[:], in1=invn_b[:],
                                op=OP.mult)                    # ||e||
        # stat input [128, 2] = [row-sum norms | row-sum norms2]; gpsimd
        # all-reduces it across partitions so every partition sees the
        # batch sums (no PE matmul, minimal ucode work).
        stat_in = constp.tile([128, 2], f32, tag="statin")
        nc.vector.reduce_sum(out=stat_in[:, 0:1], in_=norms_b[:], axis=X)
        nc.vector.reduce_sum(out=stat_in[:, 1:2], in_=norms2_b[:], axis=X)

        # gpsimd: batch sums (all partitions), then wait for the DVE-side
        # scalar chain (hooked mid-sweep) before the margin polynomial.
        stat_sums = constp.tile([128, 2], f32, tag="statsums")
        gp.partition_all_reduce(stat_sums[:], stat_in[:], channels=128,
                                reduce_op=bass_isa.ReduceOp.add)

        # scalar chain results, all computed 128-partition-redundant
        scal = smallp.tile([128, 4], f32, tag="scal")

        def emit_stats_dve():
            # mean = S1/N ; var = (S2 - S1^2/N)/(N-1); 1/(std+H)
            sct = scrp.tile([128, 2], f32, tag="sct")
            nc.vector.tensor_tensor(out=sct[:, 0:1], in0=stat_sums[:, 0:1],
                                    in1=stat_sums[:, 0:1], op=OP.mult)  # S1^2
            nc.vector.tensor_scalar_mul(sct[:, 0:1], sct[:, 0:1], 1.0 / N)
            nc.vector.tensor_tensor(out=sct[:, 0:1], in0=stat_sums[:, 1:2],
                                    in1=sct[:, 0:1], op=OP.subtract)
            nc.vector.tensor_scalar_mul(sct[:, 0:1], sct[:, 0:1],
                                        1.0 / (N - 1))         # var
            nc.vector.tensor_scalar_mul(scal[:, 0:1], stat_sums[:, 0:1],
                                        1.0 / N)               # mean
            rsqrt(nc.vector, sct[:, 0:1], sct[:, 1:2], scal[:, 2:3])  # 1/std
            nc.vector.tensor_tensor(out=sct[:, 1:2], in0=sct[:, 0:1],
                                    in1=sct[:, 1:2], op=OP.mult)     # std
            nc.vector.tensor_scalar_add(sct[:, 1:2], sct[:, 1:2], HCONST)
            nc.vector.reciprocal(out=scal[:, 1:2], in_=sct[:, 1:2])  # 1/(std+H)

        # ---- label/margin path on GpSimd (idle during the sweep) ----
        ms_b = smallp.tile([128, NTILE], f32, tag="msb")
        m_b = smallp.tile([128, NTILE], f32, tag="mb")
        u_b = smallp.tile([128, NTILE], f32, tag="ub")
        sin_b = smallp.tile([128, NTILE], f32, tag="sinb")
        cos_b = smallp.tile([128, NTILE], f32, tag="cosb")
        dots_b = smallp.tile([128, NTILE], f32, tag="dots")
        cost_b = smallp.tile([128, NTILE], f32, tag="cost")
        x2_b = smallp.tile([128, NTILE], f32, tag="x2b")
        rt_b = smallp.tile([128, NTILE], f32, tag="rtb")
        et_b = smallp.tile([128, NTILE], f32, tag="etb")
        em_b = smallp.tile([128, NTILE], f32, tag="emb2")
        costm_b = smallp.tile([128, NTILE], f32, tag="costm")
        corr_b = smallp.tile([128, NTILE], f32, tag="corrb")
        lab_b = smallp.tile([128, NTILE], f32, tag="labb")
        dscr = [scrp.tile([128, D], f32, name=f"gsq{j}", tag=f"gsq{j}")
                for j in range(NTILE)]

        def emit_label_gp():
            # margin scaler -> m, sin(m), cos(m) (poly; mult/add only)
            gp.tensor_tensor(out=ms_b[:], in0=norms_b[:],
                             in1=scal[:, 0:1].to_broadcast([128, NTILE]),
                             op=OP.subtract)
            gp.tensor_tensor(out=ms_b[:], in0=ms_b[:],
                             in1=scal[:, 1:2].to_broadcast([128, NTILE]),
                             op=OP.mult)
            gp.tensor_scalar_min(ms_b[:], ms_b[:], 1.0)
            gp.tensor_scalar_max(ms_b[:], ms_b[:], -1.0)
            gp.tensor_scalar(out=m_b[:], in0=ms_b[:], scalar1=MARGIN,
                             scalar2=MARGIN, op0=OP.mult, op1=OP.add)
            gp.tensor_tensor(out=u_b[:], in0=m_b[:], in1=m_b[:], op=OP.mult)
            gp.tensor_scalar(out=sin_b[:], in0=u_b[:], scalar1=1.0 / 120,
                             scalar2=-1.0 / 6, op0=OP.mult, op1=OP.add)
            gp.tensor_tensor(out=sin_b[:], in0=sin_b[:], in1=u_b[:],
                             op=OP.mult)
            gp.tensor_scalar_add(sin_b[:], sin_b[:], 1.0)
            gp.tensor_tensor(out=sin_b[:], in0=sin_b[:], in1=m_b[:],
                             op=OP.mult)
            gp.tensor_scalar(out=cos_b[:], in0=u_b[:], scalar1=-1.0 / 720,
                             scalar2=1.0 / 24, op0=OP.mult, op1=OP.add)
            gp.tensor_tensor(out=cos_b[:], in0=cos_b[:], in1=u_b[:],
                             op=OP.mult)
            gp.tensor_scalar_add(cos_b[:], cos_b[:], -0.5)
            gp.tensor_tensor(out=cos_b[:], in0=cos_b[:], in1=u_b[:],
                             op=OP.mult)
            gp.tensor_scalar_add(cos_b[:], cos_b[:], 1.0)

        def emit_dots_dve(j):
            nc.vector.reduce_sum(out=dots_b[:, j:j + 1], in_=dscr[j][:],
                                 axis=X)

        def emit_cost_gp():
            # cos_t = dots/||e||, clamped; rt = sqrt(1-c^2) via series in c^2
            gp.tensor_tensor(out=cost_b[:], in0=dots_b[:], in1=invn_b[:],
                             op=OP.mult)
            gp.tensor_scalar_min(cost_b[:], cost_b[:], 1.0)
            gp.tensor_scalar_max(cost_b[:], cost_b[:], -1.0)
            gp.tensor_tensor(out=x2_b[:], in0=cost_b[:], in1=cost_b[:],
                             op=OP.mult)
            gp.tensor_scalar(out=rt_b[:], in0=x2_b[:], scalar1=5.0 / 128,
                             scalar2=1.0 / 16, op0=OP.mult, op1=OP.add)
            gp.tensor_tensor(out=rt_b[:], in0=rt_b[:], in1=x2_b[:],
                             op=OP.mult)
            gp.tensor_scalar_add(rt_b[:], rt_b[:], 1.0 / 8)
            gp.tensor_tensor(out=rt_b[:], in0=rt_b[:], in1=x2_b[:],
                             op=OP.mult)
            gp.tensor_scalar_add(rt_b[:], rt_b[:], 0.5)
            gp.tensor_tensor(out=rt_b[:], in0=rt_b[:], in1=x2_b[:],
                             op=OP.mult)
            gp.tensor_scalar(out=rt_b[:], in0=rt_b[:], scalar1=-1.0,
                             scalar2=1.0, op0=OP.mult, op1=OP.add)

        def emit_label_act_a():
            nc.scalar.activation(et_b[:], cost_b[:], AF.Exp, bias=-30.0,
                                 scale=SCALEC)

        def emit_label_gp_b():
            gp.tensor_tensor(out=costm_b[:], in0=cost_b[:], in1=cos_b[:],
                             op=OP.mult)
            gp.tensor_tensor(out=rt_b[:], in0=rt_b[:], in1=sin_b[:],
                             op=OP.mult)
            gp.tensor_tensor(out=costm_b[:], in0=costm_b[:], in1=rt_b[:],
                             op=OP.subtract)
            gp.tensor_scalar_mul(lab_b[:], costm_b[:], SCALEC)
            gp.tensor_tensor(out=lab_b[:], in0=lab_b[:], in1=valid_sb[:],
                             op=OP.mult)

        def emit_label_act_c():
            nc.scalar.activation(em_b[:], costm_b[:], AF.Exp, bias=-30.0,
                                 scale=SCALEC)

        def emit_label_gp_d():
            gp.tensor_tensor(out=corr_b[:], in0=em_b[:], in1=et_b[:],
                             op=OP.subtract)
            gp.tensor_tensor(out=corr_b[:], in0=corr_b[:], in1=valid_sb[:],
                             op=OP.mult)

        # elementwise e*w products on gp (inputs already emitted above)
        for j in range(NTILE):
            gp.tensor_tensor(out=dscr[j][:], in0=emb_t[j][:],
                             in1=wlab_t[j][:], op=OP.mult)

        # ---- main sweep: g outer (DMA streaming order), j inner ----
        sums = constp.tile([128, NTILE * NGRP * 2], f32, tag="sums")
        nc.vector.memset(sums[:], 0.0)
        gidx = 0
        for g in range(NGRP):
            c0, w = grp_range(g)
            nsub = (w + SUB - 1) // SUB
            for j in range(NTILE):
                if gidx == 10:
                    emit_stats_dve()
                elif gidx in (11, 12, 13, 14):
                    emit_dots_dve(gidx - 11)
                elif gidx == 15:
                    emit_label_gp()
                    emit_cost_gp()
                elif gidx == 19:
                    emit_label_act_a()
                    emit_label_gp_b()
                elif gidx == 21:
                    emit_label_act_c()
                elif gidx == 22:
                    emit_label_gp_d()
                psA = (pmain.tile([128, ASPLIT], f32, name="psA",
                                  tag="psA")
                       if w > ASPLIT else None)
                psB = pmain.tile([128, GRP - ASPLIT], f32, name="psB",
                                 tag="psB")
                ragged = (w <= ASPLIT)
                for kk in range(2):
                    for s in range(nsub):
                        ws = min(SUB, w - s * SUB)
                        if ragged or s * SUB >= ASPLIT:
                            off = 0 if ragged else s * SUB - ASPLIT
                            dst = psB[:, off:off + ws]
                        else:
                            dst = psA[:, s * SUB:s * SUB + ws]
                        nc.tensor.matmul(
                            out=dst,
                            lhsT=embT8[:, 2 * kk:2 * kk + 2,
                                       j * 128:(j + 1) * 128],
                            rhs=w8[:, kk, :, c0 + s * SUB:c0 + s * SUB + ws],
                            perf_mode=mybir.MatmulPerfMode.DoubleRow,
                            start=(kk == 0), stop=(kk == 1))
                base = (j * NGRP + g) * 2
                if not ragged:
                    ex = actp.tile([128, ASPLIT], bf16, tag="ex")
                    nc.scalar.activation(ex[:], psA[:], AF.Exp,
                                         bias=-30.0, scale=S30,
                                         accum_out=sums[:, base:base + 1])
                wd = w - ASPLIT if not ragged else w
                ti = actp.tile([128, GRP - ASPLIT], i32, tag="ti")
                nc.vector.tensor_scalar(out=ti[:, 0:wd],
                                        in0=psB[:, 0:wd],
                                        scalar1=DVE_A, scalar2=DVE_B,
                                        op0=OP.mult, op1=OP.add)
                nc.vector.reduce_sum(
                    out=sums[:, base + 1:base + 2],
                    in_=ti[:, 0:wd].bitcast(f32),
                    axis=X)
                gidx += 1

        # ---- per-sample totals (+ label correction) + final collective ----
        stot = smallp.tile([128, 2 * NTILE], f32, tag="stot")
        for j in range(NTILE):
            nc.vector.reduce_sum(out=stot[:, j:j + 1],
                                 in_=sums[:, j * NGRP * 2:(j + 1) * NGRP * 2],
                                 axis=X)
        nc.vector.tensor_tensor(out=stot[:, 0:NTILE], in0=stot[:, 0:NTILE],
                                in1=corr_b[:], op=OP.add)
        nc.vector.tensor_copy(out=stot[:, NTILE:2 * NTILE], in_=lab_b[:])
        cc2_in = dramp.tile([128, 2 * NTILE], f32, tag="cc2in")
        cc2_out = dramp.tile([128, 2 * NTILE], f32, tag="cc2out")
        nc.sync.dma_start(out=cc2_in[:], in_=stot[:])
        gp.collective_compute(
            "AllReduce", mybir.AluOpType.add,
            replica_groups=[list(range(NCORES))],
            ins=[cc2_in.opt()], outs=[cc2_out.opt()])
        cc2_res = smallp.tile([128, 2 * NTILE], f32, tag="cc2res")
        nc.sync.dma_start(out=cc2_res[:], in_=cc2_out[:])

        lse_b = smallp.tile([128, NTILE], f32, tag="lseb")
        nc.scalar.activation(lse_b[:], cc2_res[:, 0:NTILE], AF.Ln, scale=EXP30)
        nc.vector.tensor_tensor(out=lse_b[:], in0=lse_b[:],
                                in1=cc2_res[:, NTILE:2 * NTILE],
                                op=OP.subtract)
        part = smallp.tile([128, 1], f32, tag="part")
        nc.vector.reduce_sum(out=part[:], in_=lse_b[:], axis=X)
        ps_l = pmain.tile([1, 1], f32, tag="psB")
        nc.tensor.matmul(out=ps_l[:], lhsT=ones_f[:], rhs=part[:],
                         start=True, stop=True)
        loss_sb = smallp.tile([1, 1], f32, tag="loss")
        nc.scalar.mul(loss_sb[:], ps_l[:], 1.0 / N)
        nc.sync.dma_start(out=out_d[:, :], in_=loss_sb[:])

    nc.finalize()
    return nc


def _host_prep(embeddings, labels, weight):
    import ml_dtypes
    emb = np.ascontiguousarray(embeddings, dtype=np.float32)
    w = np.ascontiguousarray(weight, dtype=np.float32)
    lab = np.asarray(labels).astype(np.int64)
    # normalize rows once for the full weight matrix
    wn = np.sqrt((w * w).sum(axis=1, keepdims=True))
    wu = w / wn
    # k-major fp8 layout for the whole matrix: [128(p), 2(kk), 2(o), C]
    # with k = kk*256 + o*128 + p
    wt8_full = np.ascontiguousarray(
        (wu.T * np.float32(FP8S)).reshape(2, 2, 128, C).transpose(2, 0, 1, 3)
    ).astype(ml_dtypes.float8_e4m3)
    ident_bf = np.eye(128, dtype=ml_dtypes.bfloat16)
    ones_f = np.ones((128, 1), dtype=np.float32)
    in_maps = []
    for core in range(NCORES):
        lab_loc = lab - core * CLOC
        valid = ((lab_loc >= 0) & (lab_loc < CLOC)).astype(np.float32)
        idx = np.clip(lab_loc, 0, CLOC - 1).astype(np.int32)
        in_maps.append({
            "wt8": np.ascontiguousarray(
                wt8_full[:, :, :, core * CLOC:(core + 1) * CLOC]),
            "wrows": np.ascontiguousarray(wu[core * CLOC:(core + 1) * CLOC]),
            "emb": emb,
            "labidx": np.ascontiguousarray(idx.reshape(NTILE, 128).T),
            "valid": np.ascontiguousarray(valid.reshape(NTILE, 128).T),
            "identbf": ident_bf,
            "onesf": ones_f,
        })
    return in_maps


def run(embeddings, labels, weight, trace=False):
    from concourse import bass_utils
    if "nc" not in _cache:
        _cache["nc"] = _build()
    in_maps = _host_prep(embeddings, labels, weight)
    res = bass_utils.run_bass_kernel_spmd(
        _cache["nc"], in_maps, core_ids=list(range(NCORES)), trace=trace)
    out = np.asarray(res.results[0]["out"], dtype=np.float32).reshape(())
    return out, res


def kernel(embeddings, labels, weight):
    out, _ = run(embeddings, labels, weight, trace=False)
    return out


# revision 39
# speedup vs baseline: 1.1473x; 1.1473x over previous
"""AdaFace loss on 8 TRN2 NeuronCores — class-parallel margin softmax.

Sharding: class dim split 12500/core. Host pre-normalizes weight rows and
casts to fp8 in k-major DoubleRow layout [128, 2(kk), 2(o), 12500]; the
device streams W from HBM (6.4MB/core) via group-aligned chunks on the
sync queue. The matmul keeps the (transposed, normalized, fp8) embeddings
stationary and streams W: psum[128 batch, 2048 classes] accumulates K=512
in 2 DoubleRow matmuls per 512-class sub-chunk. Each psum group is
consumed by ACT (exp with accum_out -> per-sample sumexp partials) and DVE
(Schraudolph fast-exp on the tail columns) in parallel.

The margin/label path is kept off the sweep engines: batch-norm stats use
gpsimd partition_all_reduce (no PE matmul), the margin polynomial and the
label-cosine chain run on gpsimd from an f32 row gather, and the few ops
that must touch DVE/ACT are emitted mid-sweep at points those engines
reach only after the inputs are ready (so the in-order queues never
stall). A tiny dummy AllReduce fires at t~0 to absorb the cross-core
rendezvous + first-collective setup; the single data AllReduce at the end
carries sumexp+corr and label logits together.
"""
import math
import numpy as np

NCORES = 8
C, D, N = 100000, 512, 512
CLOC = C // NCORES            # 12500
SUB = 512                     # classes per matmul / psum bank
GRP = 4 * SUB                 # classes per psum tile (4 banks)
NGRP_FULL = CLOC // GRP       # 6 full groups
GRP_LAST = CLOC - NGRP_FULL * GRP   # 212
NGRP = NGRP_FULL + 1          # 7
NTILE = N // 128              # 4 batch tiles
SCALEC = 30.0
MARGIN = 0.4
HCONST = 0.333
FP8S = 16.0                   # fp8 scaling for both operands
S30 = SCALEC / (FP8S * FP8S)  # activation scale: psum = 256*cos
EXP30 = float(np.exp(np.float32(30.0)))
# Schraudolph fast-exp: exp(y) ~= bitcast_f32(int(y*FEA + FEB)); for the
# DVE-consumed columns y = S30*psum - 30, so i = psum*(FEA*S30) + (FEB-30*FEA)
FEA = 12102203.161561485      # 2^23/ln(2)
FEB = 1064866805.0
DVE_A = FEA * S30
DVE_B = FEB - 30.0 * FEA
ASPLIT = 1536                 # cols per group on ACT (3 psum banks); rest on DVE

_cache = {}


def _build():
    import concourse.bass as bass
    import concourse.bacc as bacc
    import concourse.mybir as mybir
    import concourse.tile as tile
    import concourse.bass_isa as bass_isa
    from contextlib import ExitStack

    f32 = mybir.dt.float32
    bf16 = mybir.dt.bfloat16
    fp8 = mybir.dt.float8e4
    i32 = mybir.dt.int32
    AF = mybir.ActivationFunctionType
    OP = mybir.AluOpType
    X = mybir.AxisListType.X

    nc = bacc.Bacc("TRN2", target_bir_lowering=False, debug=False,
                   num_devices=NCORES)
    _c30 = nc.alloc_sbuf_tensor("const-f32-neg30", [128, 1], f32)
    nc.gpsimd.memset(_c30.ap(), -30.0)
    nc.const_aps.aps[(f32, -30.0)] = _c30.ap()
    nc.all_engine_barrier()

    wt8_d = nc.dram_tensor("wt8", [128, 2, 2, CLOC], fp8, kind="ExternalInput")
    wrows_d = nc.dram_tensor("wrows", [CLOC, D], f32, kind="ExternalInput")
    emb_d = nc.dram_tensor("emb", [N, D], f32, kind="ExternalInput")
    labidx_d = nc.dram_tensor("labidx", [128, NTILE], i32, kind="ExternalInput")
    valid_d = nc.dram_tensor("valid", [128, NTILE], f32, kind="ExternalInput")
    identbf_d = nc.dram_tensor("identbf", [128, 128], bf16, kind="ExternalInput")
    onesf_d = nc.dram_tensor("onesf", [128, 1], f32, kind="ExternalInput")
    out_d = nc.dram_tensor("out", [1, 1], f32, kind="ExternalOutput")

    def grp_range(g):
        c0 = g * GRP
        return c0, (GRP if g < NGRP_FULL else GRP_LAST)

    with tile.TileContext(nc) as tc, ExitStack() as ctx:
        constp = ctx.enter_context(tc.tile_pool(name="const", bufs=1))
        scrp = ctx.enter_context(tc.tile_pool(name="scratch", bufs=2))
        actp = ctx.enter_context(tc.tile_pool(name="actout", bufs=2))
        smallp = ctx.enter_context(tc.tile_pool(name="small", bufs=2))
        pmain = ctx.enter_context(tc.tile_pool(name="pmain", bufs=2, space="PSUM"))
        dramp = ctx.enter_context(tc.tile_pool(name="dram", bufs=1, space="DRAM"))

        gp = nc.gpsimd

        # ---- sync queue (hardware DGE, fast): rendezvous-AR input first,
        # tiny consts, embeddings, then the W chunks. The gpsimd
        # software-DGE queue is far too slow for any of these.
        warm_sb = smallp.tile([128, 1], f32, tag="warm")
        nc.vector.memset(warm_sb[:], 0.0)
        warm_in = dramp.tile([128, 1], f32, tag="warmin")
        warm_out = dramp.tile([128, 1], f32, tag="warmout")
        nc.sync.dma_start(out=warm_in[:], in_=warm_sb[:])
        idx_sb = constp.tile([128, NTILE], i32, tag="idx")
        nc.sync.dma_start(out=idx_sb[:], in_=labidx_d[:, :])
        ident_bf = constp.tile([128, 128], bf16, tag="identbf")
        nc.sync.dma_start(out=ident_bf[:], in_=identbf_d[:, :])
        ones_f = constp.tile([128, 1], f32, tag="onesf")
        nc.sync.dma_start(out=ones_f[:], in_=onesf_d[:, :])
        valid_sb = constp.tile([128, NTILE], f32, tag="valid")
        nc.sync.dma_start(out=valid_sb[:], in_=valid_d[:, :])
        emb_all = constp.tile([128, NTILE, D], f32, tag="emball")
        nc.sync.dma_start(
            out=emb_all[:],
            in_=emb_d[:, :].rearrange("(j p) d -> p j d", p=128))
        emb_t = [emb_all[:, j, :] for j in range(NTILE)]

        w8 = constp.tile([128, 2, 2, CLOC], fp8, tag="w8")
        for g in range(NGRP):
            c0, w = grp_range(g)
            nc.sync.dma_start(out=w8[:, :, :, c0:c0 + w],
                              in_=wt8_d[:, :, :, c0:c0 + w])

        # ---- gpsimd: dummy rendezvous AR trigger, then the label gather
        gp.collective_compute(
            "AllReduce", mybir.AluOpType.add,
            replica_groups=[list(range(NCORES))],
            ins=[warm_in.opt()], outs=[warm_out.opt()])

        wlab_t = []
        for j in range(NTILE):
            wl = constp.tile([128, D], f32, tag=f"wlab{j}")
            gp.indirect_dma_start(
                out=wl[:], out_offset=None, in_=wrows_d[:, :],
                in_offset=bass.IndirectOffsetOnAxis(ap=idx_sb[:, j:j + 1],
                                                    axis=0))
            wlab_t.append(wl)

        def rsqrt(eng, x_ap, y_ap, t_ap, iters=2):
            """y = 1/sqrt(x) via bitcast seed + Newton (x > 0)."""
            xi = x_ap.bitcast(i32)
            yi = y_ap.bitcast(i32)
            eng.tensor_scalar(out=yi, in0=xi, scalar1=1, scalar2=None,
                              op0=OP.arith_shift_right)
            eng.tensor_scalar(out=yi, in0=yi, scalar1=-1,
                              scalar2=0x5f3759df, op0=OP.mult, op1=OP.add)
            for _ in range(iters):
                eng.tensor_tensor(out=t_ap, in0=x_ap, in1=y_ap, op=OP.mult)
                eng.tensor_tensor(out=t_ap, in0=t_ap, in1=y_ap, op=OP.mult)
                eng.tensor_scalar(out=t_ap, in0=t_ap, scalar1=-0.5,
                                  scalar2=1.5, op0=OP.mult, op1=OP.add)
                eng.tensor_tensor(out=y_ap, in0=y_ap, in1=t_ap, op=OP.mult)

        # ---- embedding prep (DVE), pair-phased so j0's chain starts as
        # soon as its data lands. embT8 is one [128, 4(k4), 512(n)] fp8
        # tile; each j gets 4 PE transposes into one psum tile and a single
        # strided ACT cast.
        norms2_b = constp.tile([128, NTILE], f32, tag="norms2")
        invn_b = constp.tile([128, NTILE], f32, tag="invn")
        invn16_b = constp.tile([128, NTILE], f32, tag="invn16")
        embT8 = constp.tile([128, 4, N], fp8, tag="embT8")
        for jp in range(2):
            j0, j1 = 2 * jp, 2 * jp + 1
            for j in (j0, j1):
                scr = scrp.tile([128, D], f32, tag="sq")
                nc.vector.scalar_tensor_tensor(
                    out=scr[:], in0=emb_t[j][:], scalar=1.0, in1=emb_t[j][:],
                    op0=OP.mult, op1=OP.mult, accum_out=norms2_b[:, j:j + 1])
            tmp_b = scrp.tile([128, 2], f32, tag="tmpb")
            rsqrt(nc.vector, norms2_b[:, j0:j1 + 1], invn_b[:, j0:j1 + 1],
                  tmp_b[:])
            nc.vector.tensor_scalar_mul(invn16_b[:, j0:j1 + 1],
                                        invn_b[:, j0:j1 + 1], FP8S)
            for j in (j0, j1):
                e8 = scrp.tile([128, D], bf16, tag="e8")
                nc.vector.tensor_scalar_mul(e8[:], emb_t[j][:],
                                            invn16_b[:, j:j + 1])
                pst = pmain.tile([128, 4, 128], bf16, tag="psB")
                for k4 in range(4):
                    nc.tensor.transpose(out=pst[:, k4, :],
                                        in_=e8[:, k4 * 128:(k4 + 1) * 128],
                                        identity=ident_bf[:])
                nc.scalar.copy(out=embT8[:, :, j * 128:(j + 1) * 128],
                               in_=pst[:])

        norms_b = constp.tile([128, NTILE], f32, tag="norms")
        nc.vector.tensor_tensor(out=norms_b[:], in0=norms2_b[:], in1=invn_b[:],
                                op=OP.mult)                    # ||e||
        # stat input [128, 2] = [row-sum norms | row-sum norms2]; gpsimd
        # all-reduces it across partitions so every partition sees the
        # batch sums (no PE matmul, minimal ucode work).
        stat_in = constp.tile([128, 2], f32, tag="statin")
        nc.vector.reduce_sum(out=stat_in[:, 0:1], in_=norms_b[:], axis=X)
        nc.vector.reduce_sum(out=stat_in[:, 1:2], in_=norms2_b[:], axis=X)

        # gpsimd: batch sums (all partitions), then wait for the DVE-side
        # scalar chain (hooked mid-sweep) before the margin polynomial.
        stat_sums = constp.tile([128, 2], f32, tag="statsums")
        gp.partition_all_reduce(stat_sums[:], stat_in[:], channels=128,
                                reduce_op=bass_isa.ReduceOp.add)

        # scalar chain results, all computed 128-partition-redundant
        scal = smallp.tile([128, 4], f32, tag="scal")

        def emit_stats_dve():
            # mean = S1/N ; var = (S2 - S1^2/N)/(N-1); 1/(std+H)
            sct = scrp.tile([128, 2], f32, tag="sct")
            nc.vector.tensor_tensor(out=sct[:, 0:1], in0=stat_sums[:, 0:1],
                                    in1=stat_sums[:, 0:1], op=OP.mult)  # S1^2
            nc.vector.tensor_scalar_mul(sct[:, 0:1], sct[:, 0:1], 1.0 / N)
            nc.vector.tensor_tensor(out=sct[:, 0:1], in0=stat_sums[:, 1:2],
                                    in1=sct[:, 0:1], op=OP.subtract)
            nc.vector.tensor_scalar_mul(sct[:, 0:1], sct[:, 0:1],
                                        1.0 / (N - 1))         # var
            nc.vector.tensor_scalar_mul(scal[:, 0:1], stat_sums[:, 0:1],
                                        1.0 / N)               # mean
            rsqrt(nc.vector, sct[:, 0:1], sct[:, 1:2], scal[:, 2:3])  # 1/std
            nc.vector.tensor_tensor(out=sct[:, 1:2], in0=sct[:, 0:1],
                                    in1=sct[:, 1:2], op=OP.mult)     # std
            nc.vector.tensor_scalar_add(sct[:, 1:2], sct[:, 1:2], HCONST)
            nc.vector.reciprocal(out=scal[:, 1:2], in_=sct[:, 1:2])  # 1/(std+H)

        # ---- label/margin path on GpSimd (idle during the sweep) ----
        ms_b = smallp.tile([128, NTILE], f32, tag="msb")
        m_b = smallp.tile([128, NTILE], f32, tag="mb")
        u_b = smallp.tile([128, NTILE], f32, tag="ub")
        sin_b = smallp.tile([128, NTILE], f32, tag="sinb")
        cos_b = smallp.tile([128, NTILE], f32, tag="cosb")
        dots_b = smallp.tile([128, NTILE], f32, tag="dots")
        cost_b = smallp.tile([128, NTILE], f32, tag="cost")
        x2_b = smallp.tile([128, NTILE], f32, tag="x2b")
        rt_b = smallp.tile([128, NTILE], f32, tag="rtb")
        et_b = smallp.tile([128, NTILE], f32, tag="etb")
        em_b = smallp.tile([128, NTILE], f32, tag="emb2")
        costm_b = smallp.tile([128, NTILE], f32, tag="costm")
        corr_b = smallp.tile([128, NTILE], f32, tag="corrb")
        lab_b = smallp.tile([128, NTILE], f32, tag="labb")
        dscr = [scrp.tile([128, D], f32, name=f"gsq{j}", tag=f"gsq{j}")
                for j in range(NTILE)]

        def emit_label_gp():
            # margin scaler -> m, sin(m), cos(m) (poly; mult/add only)
            gp.tensor_tensor(out=ms_b[:], in0=norms_b[:],
                             in1=scal[:, 0:1].to_broadcast([128, NTILE]),
                             op=OP.subtract)
            gp.tensor_tensor(out=ms_b[:], in0=ms_b[:],
                             in1=scal[:, 1:2].to_broadcast([128, NTILE]),
                             op=OP.mult)
            gp.tensor_scalar_min(ms_b[:], ms_b[:], 1.0)
            gp.tensor_scalar_max(ms_b[:], ms_b[:], -1.0)
            gp.tensor_scalar(out=m_b[:], in0=ms_b[:], scalar1=MARGIN,
                             scalar2=MARGIN, op0=OP.mult, op1=OP.add)
            gp.tensor_tensor(out=u_b[:], in0=m_b[:], in1=m_b[:], op=OP.mult)
            gp.tensor_scalar(out=sin_b[:], in0=u_b[:], scalar1=1.0 / 120,
                             scalar2=-1.0 / 6, op0=OP.mult, op1=OP.add)
            gp.tensor_tensor(out=sin_b[:], in0=sin_b[:], in1=u_b[:],
                             op=OP.mult)
            gp.tensor_scalar_add(sin_b[:], sin_b[:], 1.0)
            gp.tensor_tensor(out=sin_b[:], in0=sin_b[:], in1=m_b[:],
                             op=OP.mult)
            gp.tensor_scalar(out=cos_b[:], in0=u_b[:], scalar1=-1.0 / 720,
                             scalar2=1.0 / 24, op0=OP.mult, op1=OP.add)
            gp.tensor_tensor(out=cos_b[:], in0=cos_b[:], in1=u_b[:],
                             op=OP.mult)
            gp.tensor_scalar_add(cos_b[:], cos_b[:], -0.5)
            gp.tensor_tensor(out=cos_b[:], in0=cos_b[:], in1=u_b[:],
                             op=OP.mult)
            gp.tensor_scalar_add(cos_b[:], cos_b[:], 1.0)

        def emit_dots_dve(j):
            nc.vector.reduce_sum(out=dots_b[:, j:j + 1], in_=dscr[j][:],
                                 axis=X)

        def emit_cost_gp():
            # cos_t = dots/||e||, clamped; rt = sqrt(1-c^2) via series in c^2
            gp.tensor_tensor(out=cost_b[:], in0=dots_b[:], in1=invn_b[:],
                             op=OP.mult)
            gp.tensor_scalar_min(cost_b[:], cost_b[:], 1.0)
            gp.tensor_scalar_max(cost_b[:], cost_b[:], -1.0)
            gp.tensor_tensor(out=x2_b[:], in0=cost_b[:], in1=cost_b[:],
                             op=OP.mult)
            gp.tensor_scalar(out=rt_b[:], in0=x2_b[:], scalar1=5.0 / 128,
                             scalar2=1.0 / 16, op0=OP.mult, op1=OP.add)
            gp.tensor_tensor(out=rt_b[:], in0=rt_b[:], in1=x2_b[:],
                             op=OP.mult)
            gp.tensor_scalar_add(rt_b[:], rt_b[:], 1.0 / 8)
            gp.tensor_tensor(out=rt_b[:], in0=rt_b[:], in1=x2_b[:],
                             op=OP.mult)
            gp.tensor_scalar_add(rt_b[:], rt_b[:], 0.5)
            gp.tensor_tensor(out=rt_b[:], in0=rt_b[:], in1=x2_b[:],
                             op=OP.mult)
            gp.tensor_scalar(out=rt_b[:], in0=rt_b[:], scalar1=-1.0,
                             scalar2=1.0, op0=OP.mult, op1=OP.add)

        def emit_label_act_a():
            nc.scalar.activation(et_b[:], cost_b[:], AF.Exp, bias=-30.0,
                                 scale=SCALEC)

        def emit_label_gp_b():
            gp.tensor_tensor(out=costm_b[:], in0=cost_b[:], in1=cos_b[:],
                             op=OP.mult)
            gp.tensor_tensor(out=rt_b[:], in0=rt_b[:], in1=sin_b[:],
                             op=OP.mult)
            gp.tensor_tensor(out=costm_b[:], in0=costm_b[:], in1=rt_b[:],
                             op=OP.subtract)
            gp.tensor_scalar_mul(lab_b[:], costm_b[:], SCALEC)
            gp.tensor_tensor(out=lab_b[:], in0=lab_b[:], in1=valid_sb[:],
                             op=OP.mult)

        def emit_label_act_c():
            nc.scalar.activation(em_b[:], costm_b[:], AF.Exp, bias=-30.0,
                                 scale=SCALEC)

        def emit_label_gp_d():
            gp.tensor_tensor(out=corr_b[:], in0=em_b[:], in1=et_b[:],
                             op=OP.subtract)
            gp.tensor_tensor(out=corr_b[:], in0=corr_b[:], in1=valid_sb[:],
                             op=OP.mult)

        # elementwise e*w products on gp (inputs already emitted above)
        for j in range(NTILE):
            gp.tensor_tensor(out=dscr[j][:], in0=emb_t[j][:],
                             in1=wlab_t[j][:], op=OP.mult)

        # ---- main sweep: g outer (DMA streaming order), j inner ----
        sums = constp.tile([128, NTILE * NGRP * 2], f32, tag="sums")
        nc.vector.memset(sums[:], 0.0)
        gidx = 0
        for g in range(NGRP):
            c0, w = grp_range(g)
            nsub = (w + SUB - 1) // SUB
            for j in range(NTILE):
                if gidx == 10:
                    emit_stats_dve()
                elif gidx in (11, 12, 13, 14):
                    emit_dots_dve(gidx - 11)
                elif gidx == 15:
                    emit_label_gp()
                    emit_cost_gp()
                elif gidx == 19:
                    emit_label_act_a()
                    emit_label_gp_b()
                elif gidx == 21:
                    emit_label_act_c()
                elif gidx == 22:
                    emit_label_gp_d()
                psA = (pmain.tile([128, ASPLIT], f32, name="psA",
                                  tag="psA")
                       if w > ASPLIT else None)
                psB = pmain.tile([128, GRP - ASPLIT], f32, name="psB",
                                 tag="psB")
                ragged = (w <= ASPLIT)
                for kk in range(2):
                    for s in range(nsub):
                        ws = min(SUB, w - s * SUB)
                        if ragged or s * SUB >= ASPLIT:
                            off = 0 if ragged else s * SUB - ASPLIT
                            dst = psB[:, off:off + ws]
                        else:
                            dst = psA[:, s * SUB:s * SUB + ws]
                        nc.tensor.matmul(
                            out=dst,
                            lhsT=embT8[:, 2 * kk:2 * kk + 2,
                                       j * 128:(j + 1) * 128],
                            rhs=w8[:, kk, :, c0 + s * SUB:c0 + s * SUB + ws],
                            perf_mode=mybir.MatmulPerfMode.DoubleRow,
                            start=(kk == 0), stop=(kk == 1))
                base = (j * NGRP + g) * 2
                if not ragged:
                    ex = actp.tile([128, ASPLIT], bf16, tag="ex")
                    nc.scalar.activation(ex[:], psA[:], AF.Exp,
                                         bias=-30.0, scale=S30,
                                         accum_out=sums[:, base:base + 1])
                wd = w - ASPLIT if not ragged else w
                ti = actp.tile([128, GRP - ASPLIT], i32, tag="ti")
                nc.vector.tensor_scalar(out=ti[:, 0:wd],
                                        in0=psB[:, 0:wd],
                                        scalar1=DVE_A, scalar2=DVE_B,
                                        op0=OP.mult, op1=OP.add)
                nc.vector.reduce_sum(
                    out=sums[:, base + 1:base + 2],
                    in_=ti[:, 0:wd].bitcast(f32),
                    axis=X)
                gidx += 1

        # ---- per-sample totals (+ label correction) + final collective ----
        stot = smallp.tile([128, 2 * NTILE], f32, tag="stot")
        for j in range(NTILE):
            nc.vector.reduce_sum(out=stot[:, j:j + 1],
                                 in_=sums[:, j * NGRP * 2:(j + 1) * NGRP * 2],
                                 axis=X)
        nc.vector.tensor_tensor(out=stot[:, 0:NTILE], in0=stot[:, 0:NTILE],
                                in1=corr_b[:], op=OP.add)
        nc.vector.tensor_copy(out=stot[:, NTILE:2 * NTILE], in_=lab_b[:])
        cc2_in = dramp.tile([128, 2 * NTILE], f32, tag="cc2in")
        cc2_out = dramp.tile([128, 2 * NTILE], f32, tag="cc2out")
        nc.sync.dma_start(out=cc2_in[:], in_=stot[:])
        gp.collective_compute(
            "AllReduce", mybir.AluOpType.add,
            replica_groups=[list(range(NCORES))],
            ins=[cc2_in.opt()], outs=[cc2_out.opt()])
        cc2_res = smallp.tile([128, 2 * NTILE], f32, tag="cc2res")
        nc.sync.dma_start(out=cc2_res[:], in_=cc2_out[:])

        lse_b = smallp.tile([128, NTILE], f32, tag="lseb")
        nc.scalar.activation(lse_b[:], cc2_res[:, 0:NTILE], AF.Ln, scale=EXP30)
        nc.vector.tensor_tensor(out=lse_b[:], in0=lse_b[:],
                                in1=cc2_res[:, NTILE:2 * NTILE],
                                op=OP.subtract)
        part = smallp.tile([128, 1], f32, tag="part")
        nc.vector.reduce_sum(out=part[:], in_=lse_b[:], axis=X)
        ps_l = pmain.tile([1, 1], f32, tag="psB")
        nc.tensor.matmul(out=ps_l[:], lhsT=ones_f[:], rhs=part[:],
                         start=True, stop=True)
        loss_sb = smallp.tile([1, 1], f32, tag="loss")
        nc.scalar.mul(loss_sb[:], ps_l[:], 1.0 / N)
        nc.sync.dma_start(out=out_d[:, :], in_=loss_sb[:])

    nc.finalize()
    return nc


def _host_prep(embeddings, labels, weight):
    import ml_dtypes
    emb = np.ascontiguousarray(embeddings, dtype=np.float32)
    w = np.ascontiguousarray(weight, dtype=np.float32)
    lab = np.asarray(labels).astype(np.int64)
    # normalize rows once for the full weight matrix
    wn = np.sqrt((w * w).sum(axis=1, keepdims=True))
    wu = w / wn
    # k-major fp8 layout for the whole matrix: [128(p), 2(kk), 2(o), C]
    # with k = kk*256 + o*128 + p
    wt8_full = np.ascontiguousarray(
        (wu.T * np.float32(FP8S)).reshape(2, 2, 128, C).transpose(2, 0, 1, 3)
    ).astype(ml_dtypes.float8_e4m3)
    ident_bf = np.eye(128, dtype=ml_dtypes.bfloat16)
    ones_f = np.ones((128, 1), dtype=np.float32)
    in_maps = []
    for core in range(NCORES):
        lab_loc = lab - core * CLOC
        valid = ((lab_loc >= 0) & (lab_loc < CLOC)).astype(np.float32)
        idx = np.clip(lab_loc, 0, CLOC - 1).astype(np.int32)
        in_maps.append({
            "wt8": np.ascontiguousarray(
                wt8_full[:, :, :, core * CLOC:(core + 1) * CLOC]),
            "wrows": np.ascontiguousarray(wu[core * CLOC:(core + 1) * CLOC]),
            "emb": emb,
            "labidx": np.ascontiguousarray(idx.reshape(NTILE, 128).T),
            "valid": np.ascontiguousarray(valid.reshape(NTILE, 128).T),
            "identbf": ident_bf,
            "onesf": ones_f,
        })
    return in_maps


def run(embeddings, labels, weight, trace=False):
    from concourse import bass_utils
    if "nc" not in _cache:
        _cache["nc"] = _build()
    in_maps = _host_prep(embeddings, labels, weight)
    res = bass_utils.run_bass_kernel_spmd(
        _cache["nc"], in_maps, core_ids=list(range(NCORES)), trace=trace)
    out = np.asarray(res.results[0]["out"], dtype=np.float32).reshape(())
    return out, res


def kernel(embeddings, labels, weight):
    out, _ = run(embeddings, labels, weight, trace=False)
    return out


# revision 40
# speedup vs baseline: 1.1616x; 1.0125x over previous
"""AdaFace loss on 8 TRN2 NeuronCores — class-parallel margin softmax.

Sharding: class dim split 12500/core. Host pre-normalizes weight rows and
casts to fp8 in k-major DoubleRow layout [128, 2(kk), 2(o), 12500]; the
device streams W from HBM (6.4MB/core) via group-aligned chunks on the
sync queue. The matmul keeps the (transposed, normalized, fp8) embeddings
stationary and streams W: psum[128 batch, 2048 classes] accumulates K=512
in 2 DoubleRow matmuls per 512-class sub-chunk. Each psum group is
consumed by ACT (exp with accum_out -> per-sample sumexp partials) and DVE
(Schraudolph fast-exp on the tail columns) in parallel.

The margin/label path is kept off the sweep engines: batch-norm stats use
gpsimd partition_all_reduce (no PE matmul), the margin polynomial and the
label-cosine chain run on gpsimd from an f32 row gather, and the few ops
that must touch DVE/ACT are emitted mid-sweep at points those engines
reach only after the inputs are ready (so the in-order queues never
stall). A tiny dummy AllReduce fires at t~0 to absorb the cross-core
rendezvous + first-collective setup; the single data AllReduce at the end
carries sumexp+corr and label logits together.
"""
import math
import numpy as np

NCORES = 8
C, D, N = 100000, 512, 512
CLOC = C // NCORES            # 12500
SUB = 512                     # classes per matmul / psum bank
GRP = 4 * SUB                 # classes per psum tile (4 banks)
NGRP_FULL = CLOC // GRP       # 6 full groups
GRP_LAST = CLOC - NGRP_FULL * GRP   # 212
NGRP = NGRP_FULL + 1          # 7
NTILE = N // 128              # 4 batch tiles
SCALEC = 30.0
MARGIN = 0.4
HCONST = 0.333
FP8S = 16.0                   # fp8 scaling for both operands
S30 = SCALEC / (FP8S * FP8S)  # activation scale: psum = 256*cos
EXP30 = float(np.exp(np.float32(30.0)))
# Schraudolph fast-exp: exp(y) ~= bitcast_f32(int(y*FEA + FEB)); for the
# DVE-consumed columns y = S30*psum - 30, so i = psum*(FEA*S30) + (FEB-30*FEA)
FEA = 12102203.161561485      # 2^23/ln(2)
FEB = 1064866805.0
DVE_A = FEA * S30
DVE_B = FEB - 30.0 * FEA
ASPLIT = 1536                 # cols per group on ACT (3 psum banks); rest on DVE

_cache = {}


def _build():
    import concourse.bass as bass
    import concourse.bacc as bacc
    import concourse.mybir as mybir
    import concourse.tile as tile
    import concourse.bass_isa as bass_isa
    from contextlib import ExitStack

    f32 = mybir.dt.float32
    bf16 = mybir.dt.bfloat16
    fp8 = mybir.dt.float8e4
    i32 = mybir.dt.int32
    AF = mybir.ActivationFunctionType
    OP = mybir.AluOpType
    X = mybir.AxisListType.X

    nc = bacc.Bacc("TRN2", target_bir_lowering=False, debug=False,
                   num_devices=NCORES)
    _c30 = nc.alloc_sbuf_tensor("const-f32-neg30", [128, 1], f32)
    nc.gpsimd.memset(_c30.ap(), -30.0)
    nc.const_aps.aps[(f32, -30.0)] = _c30.ap()
    nc.all_engine_barrier()

    wt8_d = nc.dram_tensor("wt8", [128, 2, 2, CLOC], fp8, kind="ExternalInput")
    wrows_d = nc.dram_tensor("wrows", [CLOC, D], f32, kind="ExternalInput")
    emb_d = nc.dram_tensor("emb", [N, D], f32, kind="ExternalInput")
    labidx_d = nc.dram_tensor("labidx", [128, NTILE], i32, kind="ExternalInput")
    valid_d = nc.dram_tensor("valid", [128, NTILE], f32, kind="ExternalInput")
    identbf_d = nc.dram_tensor("identbf", [128, 128], bf16, kind="ExternalInput")
    onesf_d = nc.dram_tensor("onesf", [128, 1], f32, kind="ExternalInput")
    out_d = nc.dram_tensor("out", [1, 1], f32, kind="ExternalOutput")

    def grp_range(g):
        c0 = g * GRP
        return c0, (GRP if g < NGRP_FULL else GRP_LAST)

    with tile.TileContext(nc) as tc, ExitStack() as ctx:
        constp = ctx.enter_context(tc.tile_pool(name="const", bufs=1))
        scrp = ctx.enter_context(tc.tile_pool(name="scratch", bufs=2))
        actp = ctx.enter_context(tc.tile_pool(name="actout", bufs=2))
        smallp = ctx.enter_context(tc.tile_pool(name="small", bufs=2))
        pmain = ctx.enter_context(tc.tile_pool(name="pmain", bufs=2, space="PSUM"))
        dramp = ctx.enter_context(tc.tile_pool(name="dram", bufs=1, space="DRAM"))

        gp = nc.gpsimd

        # ---- sync queue (hardware DGE, fast): rendezvous-AR input first,
        # tiny consts, embeddings, then the W chunks. The gpsimd
        # software-DGE queue is far too slow for any of these.
        warm_sb = smallp.tile([128, 1], f32, tag="warm")
        nc.vector.memset(warm_sb[:], 0.0)
        warm_in = dramp.tile([128, 1], f32, tag="warmin")
        warm_out = dramp.tile([128, 1], f32, tag="warmout")
        nc.sync.dma_start(out=warm_in[:], in_=warm_sb[:])
        idx_sb = constp.tile([128, NTILE], i32, tag="idx")
        nc.sync.dma_start(out=idx_sb[:], in_=labidx_d[:, :])
        ident_bf = constp.tile([128, 128], bf16, tag="identbf")
        nc.sync.dma_start(out=ident_bf[:], in_=identbf_d[:, :])
        ones_f = constp.tile([128, 1], f32, tag="onesf")
        nc.sync.dma_start(out=ones_f[:], in_=onesf_d[:, :])
        valid_sb = constp.tile([128, NTILE], f32, tag="valid")
        nc.sync.dma_start(out=valid_sb[:], in_=valid_d[:, :])
        # embeddings ride the scalar-engine hardware-DGE queue so their
        # transfer warms up in parallel with the sync queue's W stream
        # (emb data gates the whole prep chain).
        emb_all = constp.tile([128, NTILE, D], f32, tag="emball")
        nc.scalar.dma_start(
            out=emb_all[:],
            in_=emb_d[:, :].rearrange("(j p) d -> p j d", p=128))
        emb_t = [emb_all[:, j, :] for j in range(NTILE)]

        w8 = constp.tile([128, 2, 2, CLOC], fp8, tag="w8")
        for g in range(NGRP):
            c0, w = grp_range(g)
            nc.sync.dma_start(out=w8[:, :, :, c0:c0 + w],
                              in_=wt8_d[:, :, :, c0:c0 + w])

        # ---- gpsimd: dummy rendezvous AR trigger, then the label gather
        gp.collective_compute(
            "AllReduce", mybir.AluOpType.add,
            replica_groups=[list(range(NCORES))],
            ins=[warm_in.opt()], outs=[warm_out.opt()])

        wlab_t = []
        for j in range(NTILE):
            wl = constp.tile([128, D], f32, tag=f"wlab{j}")
            gp.indirect_dma_start(
                out=wl[:], out_offset=None, in_=wrows_d[:, :],
                in_offset=bass.IndirectOffsetOnAxis(ap=idx_sb[:, j:j + 1],
                                                    axis=0))
            wlab_t.append(wl)

        def rsqrt(eng, x_ap, y_ap, t_ap, iters=2):
            """y = 1/sqrt(x) via bitcast seed + Newton (x > 0)."""
            xi = x_ap.bitcast(i32)
            yi = y_ap.bitcast(i32)
            eng.tensor_scalar(out=yi, in0=xi, scalar1=1, scalar2=None,
                              op0=OP.arith_shift_right)
            eng.tensor_scalar(out=yi, in0=yi, scalar1=-1,
                              scalar2=0x5f3759df, op0=OP.mult, op1=OP.add)
            for _ in range(iters):
                eng.tensor_tensor(out=t_ap, in0=x_ap, in1=y_ap, op=OP.mult)
                eng.tensor_tensor(out=t_ap, in0=t_ap, in1=y_ap, op=OP.mult)
                eng.tensor_scalar(out=t_ap, in0=t_ap, scalar1=-0.5,
                                  scalar2=1.5, op0=OP.mult, op1=OP.add)
                eng.tensor_tensor(out=y_ap, in0=y_ap, in1=t_ap, op=OP.mult)

        # ---- embedding prep (DVE), pair-phased so j0's chain starts as
        # soon as its data lands. embT8 is one [128, 4(k4), 512(n)] fp8
        # tile; each j gets 4 PE transposes into one psum tile and a single
        # strided ACT cast.
        norms2_b = constp.tile([128, NTILE], f32, tag="norms2")
        invn_b = constp.tile([128, NTILE], f32, tag="invn")
        invn16_b = constp.tile([128, NTILE], f32, tag="invn16")
        embT8 = constp.tile([128, 4, N], fp8, tag="embT8")
        for jp in range(2):
            j0, j1 = 2 * jp, 2 * jp + 1
            for j in (j0, j1):
                scr = scrp.tile([128, D], f32, tag="sq")
                nc.vector.scalar_tensor_tensor(
                    out=scr[:], in0=emb_t[j][:], scalar=1.0, in1=emb_t[j][:],
                    op0=OP.mult, op1=OP.mult, accum_out=norms2_b[:, j:j + 1])
            tmp_b = scrp.tile([128, 2], f32, tag="tmpb")
            rsqrt(nc.vector, norms2_b[:, j0:j1 + 1], invn_b[:, j0:j1 + 1],
                  tmp_b[:])
            nc.vector.tensor_scalar_mul(invn16_b[:, j0:j1 + 1],
                                        invn_b[:, j0:j1 + 1], FP8S)
            for j in (j0, j1):
                e8 = scrp.tile([128, D], bf16, tag="e8")
                nc.vector.tensor_scalar_mul(e8[:], emb_t[j][:],
                                            invn16_b[:, j:j + 1])
                pst = pmain.tile([128, 4, 128], bf16, tag="psB")
                for k4 in range(4):
                    nc.tensor.transpose(out=pst[:, k4, :],
                                        in_=e8[:, k4 * 128:(k4 + 1) * 128],
                                        identity=ident_bf[:])
                nc.scalar.copy(out=embT8[:, :, j * 128:(j + 1) * 128],
                               in_=pst[:])

        norms_b = constp.tile([128, NTILE], f32, tag="norms")
        nc.vector.tensor_tensor(out=norms_b[:], in0=norms2_b[:], in1=invn_b[:],
                                op=OP.mult)                    # ||e||
        # stat input [128, 2] = [row-sum norms | row-sum norms2]; gpsimd
        # all-reduces it across partitions so every partition sees the
        # batch sums (no PE matmul, minimal ucode work).
        stat_in = constp.tile([128, 2], f32, tag="statin")
        nc.vector.reduce_sum(out=stat_in[:, 0:1], in_=norms_b[:], axis=X)
        nc.vector.reduce_sum(out=stat_in[:, 1:2], in_=norms2_b[:], axis=X)

        # gpsimd: batch sums (all partitions), then wait for the DVE-side
        # scalar chain (hooked mid-sweep) before the margin polynomial.
        stat_sums = constp.tile([128, 2], f32, tag="statsums")
        gp.partition_all_reduce(stat_sums[:], stat_in[:], channels=128,
                                reduce_op=bass_isa.ReduceOp.add)

        # scalar chain results, all computed 128-partition-redundant
        scal = smallp.tile([128, 4], f32, tag="scal")

        def emit_stats_dve():
            # mean = S1/N ; var = (S2 - S1^2/N)/(N-1); 1/(std+H)
            sct = scrp.tile([128, 2], f32, tag="sct")
            nc.vector.tensor_tensor(out=sct[:, 0:1], in0=stat_sums[:, 0:1],
                                    in1=stat_sums[:, 0:1], op=OP.mult)  # S1^2
            nc.vector.tensor_scalar_mul(sct[:, 0:1], sct[:, 0:1], 1.0 / N)
            nc.vector.tensor_tensor(out=sct[:, 0:1], in0=stat_sums[:, 1:2],
                                    in1=sct[:, 0:1], op=OP.subtract)
            nc.vector.tensor_scalar_mul(sct[:, 0:1], sct[:, 0:1],
                                        1.0 / (N - 1))         # var
            nc.vector.tensor_scalar_mul(scal[:, 0:1], stat_sums[:, 0:1],
                                        1.0 / N)               # mean
            rsqrt(nc.vector, sct[:, 0:1], sct[:, 1:2], scal[:, 2:3])  # 1/std
            nc.vector.tensor_tensor(out=sct[:, 1:2], in0=sct[:, 0:1],
                                    in1=sct[:, 1:2], op=OP.mult)     # std
            nc.vector.tensor_scalar_add(sct[:, 1:2], sct[:, 1:2], HCONST)
            nc.vector.reciprocal(out=scal[:, 1:2], in_=sct[:, 1:2])  # 1/(std+H)

        # ---- label/margin path on GpSimd (idle during the sweep) ----
        ms_b = smallp.tile([128, NTILE], f32, tag="msb")
        m_b = smallp.tile([128, NTILE], f32, tag="mb")
        u_b = smallp.tile([128, NTILE], f32, tag="ub")
        sin_b = smallp.tile([128, NTILE], f32, tag="sinb")
        cos_b = smallp.tile([128, NTILE], f32, tag="cosb")
        dots_b = smallp.tile([128, NTILE], f32, tag="dots")
        cost_b = smallp.tile([128, NTILE], f32, tag="cost")
        x2_b = smallp.tile([128, NTILE], f32, tag="x2b")
        rt_b = smallp.tile([128, NTILE], f32, tag="rtb")
        et_b = smallp.tile([128, NTILE], f32, tag="etb")
        em_b = smallp.tile([128, NTILE], f32, tag="emb2")
        costm_b = smallp.tile([128, NTILE], f32, tag="costm")
        corr_b = smallp.tile([128, NTILE], f32, tag="corrb")
        lab_b = smallp.tile([128, NTILE], f32, tag="labb")
        dscr = [scrp.tile([128, D], f32, name=f"gsq{j}", tag=f"gsq{j}")
                for j in range(NTILE)]

        def emit_label_gp():
            # margin scaler -> m, sin(m), cos(m) (poly; mult/add only)
            gp.tensor_tensor(out=ms_b[:], in0=norms_b[:],
                             in1=scal[:, 0:1].to_broadcast([128, NTILE]),
                             op=OP.subtract)
            gp.tensor_tensor(out=ms_b[:], in0=ms_b[:],
                             in1=scal[:, 1:2].to_broadcast([128, NTILE]),
                             op=OP.mult)
            gp.tensor_scalar_min(ms_b[:], ms_b[:], 1.0)
            gp.tensor_scalar_max(ms_b[:], ms_b[:], -1.0)
            gp.tensor_scalar(out=m_b[:], in0=ms_b[:], scalar1=MARGIN,
                             scalar2=MARGIN, op0=OP.mult, op1=OP.add)
            gp.tensor_tensor(out=u_b[:], in0=m_b[:], in1=m_b[:], op=OP.mult)
            gp.tensor_scalar(out=sin_b[:], in0=u_b[:], scalar1=1.0 / 120,
                             scalar2=-1.0 / 6, op0=OP.mult, op1=OP.add)
            gp.tensor_tensor(out=sin_b[:], in0=sin_b[:], in1=u_b[:],
                             op=OP.mult)
            gp.tensor_scalar_add(sin_b[:], sin_b[:], 1.0)
            gp.tensor_tensor(out=sin_b[:], in0=sin_b[:], in1=m_b[:],
                             op=OP.mult)
            gp.tensor_scalar(out=cos_b[:], in0=u_b[:], scalar1=-1.0 / 720,
                             scalar2=1.0 / 24, op0=OP.mult, op1=OP.add)
            gp.tensor_tensor(out=cos_b[:], in0=cos_b[:], in1=u_b[:],
                             op=OP.mult)
            gp.tensor_scalar_add(cos_b[:], cos_b[:], -0.5)
            gp.tensor_tensor(out=cos_b[:], in0=cos_b[:], in1=u_b[:],
                             op=OP.mult)
            gp.tensor_scalar_add(cos_b[:], cos_b[:], 1.0)

        def emit_dots_dve(j):
            nc.vector.reduce_sum(out=dots_b[:, j:j + 1], in_=dscr[j][:],
                                 axis=X)

        def emit_cost_gp():
            # cos_t = dots/||e||, clamped; rt = sqrt(1-c^2) via series in c^2
            gp.tensor_tensor(out=cost_b[:], in0=dots_b[:], in1=invn_b[:],
                             op=OP.mult)
            gp.tensor_scalar_min(cost_b[:], cost_b[:], 1.0)
            gp.tensor_scalar_max(cost_b[:], cost_b[:], -1.0)
            gp.tensor_tensor(out=x2_b[:], in0=cost_b[:], in1=cost_b[:],
                             op=OP.mult)
            gp.tensor_scalar(out=rt_b[:], in0=x2_b[:], scalar1=5.0 / 128,
                             scalar2=1.0 / 16, op0=OP.mult, op1=OP.add)
            gp.tensor_tensor(out=rt_b[:], in0=rt_b[:], in1=x2_b[:],
                             op=OP.mult)
            gp.tensor_scalar_add(rt_b[:], rt_b[:], 1.0 / 8)
            gp.tensor_tensor(out=rt_b[:], in0=rt_b[:], in1=x2_b[:],
                             op=OP.mult)
            gp.tensor_scalar_add(rt_b[:], rt_b[:], 0.5)
            gp.tensor_tensor(out=rt_b[:], in0=rt_b[:], in1=x2_b[:],
                             op=OP.mult)
            gp.tensor_scalar(out=rt_b[:], in0=rt_b[:], scalar1=-1.0,
                             scalar2=1.0, op0=OP.mult, op1=OP.add)

        def emit_label_act_a():
            nc.scalar.activation(et_b[:], cost_b[:], AF.Exp, bias=-30.0,
                                 scale=SCALEC)

        def emit_label_gp_b():
            gp.tensor_tensor(out=costm_b[:], in0=cost_b[:], in1=cos_b[:],
                             op=OP.mult)
            gp.tensor_tensor(out=rt_b[:], in0=rt_b[:], in1=sin_b[:],
                             op=OP.mult)
            gp.tensor_tensor(out=costm_b[:], in0=costm_b[:], in1=rt_b[:],
                             op=OP.subtract)
            gp.tensor_scalar_mul(lab_b[:], costm_b[:], SCALEC)
            gp.tensor_tensor(out=lab_b[:], in0=lab_b[:], in1=valid_sb[:],
                             op=OP.mult)

        def emit_label_act_c():
            nc.scalar.activation(em_b[:], costm_b[:], AF.Exp, bias=-30.0,
                                 scale=SCALEC)

        def emit_label_gp_d():
            gp.tensor_tensor(out=corr_b[:], in0=em_b[:], in1=et_b[:],
                             op=OP.subtract)
            gp.tensor_tensor(out=corr_b[:], in0=corr_b[:], in1=valid_sb[:],
                             op=OP.mult)

        # elementwise e*w products on gp (inputs already emitted above)
        for j in range(NTILE):
            gp.tensor_tensor(out=dscr[j][:], in0=emb_t[j][:],
                             in1=wlab_t[j][:], op=OP.mult)

        # ---- main sweep: g outer (DMA streaming order), j inner ----
        sums = constp.tile([128, NTILE * NGRP * 2], f32, tag="sums")
        nc.vector.memset(sums[:], 0.0)
        gidx = 0
        for g in range(NGRP):
            c0, w = grp_range(g)
            nsub = (w + SUB - 1) // SUB
            for j in range(NTILE):
                if gidx == 10:
                    emit_stats_dve()
                elif gidx in (11, 12, 13, 14):
                    emit_dots_dve(gidx - 11)
                elif gidx == 15:
                    emit_label_gp()
                    emit_cost_gp()
                elif gidx == 19:
                    emit_label_act_a()
                    emit_label_gp_b()
                elif gidx == 21:
                    emit_label_act_c()
                elif gidx == 22:
                    emit_label_gp_d()
                psA = (pmain.tile([128, ASPLIT], f32, name="psA",
                                  tag="psA")
                       if w > ASPLIT else None)
                psB = pmain.tile([128, GRP - ASPLIT], f32, name="psB",
                                 tag="psB")
                ragged = (w <= ASPLIT)
                for kk in range(2):
                    for s in range(nsub):
                        ws = min(SUB, w - s * SUB)
                        if ragged or s * SUB >= ASPLIT:
                            off = 0 if ragged else s * SUB - ASPLIT
                            dst = psB[:, off:off + ws]
                        else:
                            dst = psA[:, s * SUB:s * SUB + ws]
                        nc.tensor.matmul(
                            out=dst,
                            lhsT=embT8[:, 2 * kk:2 * kk + 2,
                                       j * 128:(j + 1) * 128],
                            rhs=w8[:, kk, :, c0 + s * SUB:c0 + s * SUB + ws],
                            perf_mode=mybir.MatmulPerfMode.DoubleRow,
                            start=(kk == 0), stop=(kk == 1))
                base = (j * NGRP + g) * 2
                if not ragged:
                    ex = actp.tile([128, ASPLIT], bf16, tag="ex")
                    nc.scalar.activation(ex[:], psA[:], AF.Exp,
                                         bias=-30.0, scale=S30,
                                         accum_out=sums[:, base:base + 1])
                wd = w - ASPLIT if not ragged else w
                ti = actp.tile([128, GRP - ASPLIT], i32, tag="ti")
                nc.vector.tensor_scalar(out=ti[:, 0:wd],
                                        in0=psB[:, 0:wd],
                                        scalar1=DVE_A, scalar2=DVE_B,
                                        op0=OP.mult, op1=OP.add)
                nc.vector.reduce_sum(
                    out=sums[:, base + 1:base + 2],
                    in_=ti[:, 0:wd].bitcast(f32),
                    axis=X)
                gidx += 1

        # ---- per-sample totals (+ label correction) + final collective ----
        stot = smallp.tile([128, 2 * NTILE], f32, tag="stot")
        for j in range(NTILE):
            nc.vector.reduce_sum(out=stot[:, j:j + 1],
                                 in_=sums[:, j * NGRP * 2:(j + 1) * NGRP * 2],
                                 axis=X)
        nc.vector.tensor_tensor(out=stot[:, 0:NTILE], in0=stot[:, 0:NTILE],
                                in1=corr_b[:], op=OP.add)
        nc.vector.tensor_copy(out=stot[:, NTILE:2 * NTILE], in_=lab_b[:])
        cc2_in = dramp.tile([128, 2 * NTILE], f32, tag="cc2in")
        cc2_out = dramp.tile([128, 2 * NTILE], f32, tag="cc2out")
        nc.sync.dma_start(out=cc2_in[:], in_=stot[:])
        gp.collective_compute(
            "AllReduce", mybir.AluOpType.add,
            replica_groups=[list(range(NCORES))],
            ins=[cc2_in.opt()], outs=[cc2_out.opt()])
        cc2_res = smallp.tile([128, 2 * NTILE], f32, tag="cc2res")
        nc.sync.dma_start(out=cc2_res[:], in_=cc2_out[:])

        lse_b = smallp.tile([128, NTILE], f32, tag="lseb")
        nc.scalar.activation(lse_b[:], cc2_res[:, 0:NTILE], AF.Ln, scale=EXP30)
        nc.vector.tensor_tensor(out=lse_b[:], in0=lse_b[:],
                                in1=cc2_res[:, NTILE:2 * NTILE],
                                op=OP.subtract)
        part = smallp.tile([128, 1], f32, tag="part")
        nc.vector.reduce_sum(out=part[:], in_=lse_b[:], axis=X)
        ps_l = pmain.tile([1, 1], f32, tag="psB")
        nc.tensor.matmul(out=ps_l[:], lhsT=ones_f[:], rhs=part[:],
                         start=True, stop=True)
        loss_sb = smallp.tile([1, 1], f32, tag="loss")
        nc.scalar.mul(loss_sb[:], ps_l[:], 1.0 / N)
        nc.sync.dma_start(out=out_d[:, :], in_=loss_sb[:])

    nc.finalize()
    return nc


def _host_prep(embeddings, labels, weight):
    import ml_dtypes
    emb = np.ascontiguousarray(embeddings, dtype=np.float32)
    w = np.ascontiguousarray(weight, dtype=np.float32)
    lab = np.asarray(labels).astype(np.int64)
    # normalize rows once for the full weight matrix
    wn = np.sqrt((w * w).sum(axis=1, keepdims=True))
    wu = w / wn
    # k-major fp8 layout for the whole matrix: [128(p), 2(kk), 2(o), C]
    # with k = kk*256 + o*128 + p
    wt8_full = np.ascontiguousarray(
        (wu.T * np.float32(FP8S)).reshape(2, 2, 128, C).transpose(2, 0, 1, 3)
    ).astype(ml_dtypes.float8_e4m3)
    ident_bf = np.eye(128, dtype=ml_dtypes.bfloat16)
    ones_f = np.ones((128, 1), dtype=np.float32)
    in_maps = []
    for core in range(NCORES):
        lab_loc = lab - core * CLOC
        valid = ((lab_loc >= 0) & (lab_loc < CLOC)).astype(np.float32)
        idx = np.clip(lab_loc, 0, CLOC - 1).astype(np.int32)
        in_maps.append({
            "wt8": np.ascontiguousarray(
                wt8_full[:, :, :, core * CLOC:(core + 1) * CLOC]),
            "wrows": np.ascontiguousarray(wu[core * CLOC:(core + 1) * CLOC]),
            "emb": emb,
            "labidx": np.ascontiguousarray(idx.reshape(NTILE, 128).T),
            "valid": np.ascontiguousarray(valid.reshape(NTILE, 128).T),
            "identbf": ident_bf,
            "onesf": ones_f,
        })
    return in_maps


def run(embeddings, labels, weight, trace=False):
    from concourse import bass_utils
    if "nc" not in _cache:
        _cache["nc"] = _build()
    in_maps = _host_prep(embeddings, labels, weight)
    res = bass_utils.run_bass_kernel_spmd(
        _cache["nc"], in_maps, core_ids=list(range(NCORES)), trace=trace)
    out = np.asarray(res.results[0]["out"], dtype=np.float32).reshape(())
    return out, res


def kernel(embeddings, labels, weight):
    out, _ = run(embeddings, labels, weight, trace=False)
    return out
